# revision 1
# baseline (speedup 1.0000x reference)
"""CausalQueue concat kernel for Trainium2 (8 NeuronCores, SPMD).

Semantics (from the reference):
    x_past = buffer[head] if size == D else zeros_like(x)
    out    = concat((x_past, x), axis=1)          # [B, 2*C]

Strategy: the op is embarrassingly data-parallel over the batch axis.
The host slices the single needed ring-buffer row `buffer[head]` (4 MB
out of the 512 MB buffer) and shards batch rows across the 8 cores
(256 rows each). Each core performs two DRAM->DRAM DMAs writing the
two column-halves of its output shard. Per-core HBM traffic: 1 MB read
+ 1 MB write.
"""

import numpy as np

B, C, D = 2048, 512, 128
N_CORES = 8
ROWS = B // N_CORES  # 256

_CACHE: dict = {}


def _build_nc():
    import concourse.bass as bass
    import concourse.mybir as mybir

    nc = bass.Bass()
    xp = nc.declare_dram_parameter("xp", [ROWS, C], mybir.dt.float32, isOutput=False)
    x = nc.declare_dram_parameter("x", [ROWS, C], mybir.dt.float32, isOutput=False)
    out = nc.declare_dram_parameter(
        "out", [ROWS, 2 * C], mybir.dt.float32, isOutput=True
    )

    with (
        nc.Block() as block,
        nc.semaphore("dma_sem") as dma_sem,
    ):

        @block.sync
        def _(sync):
            sync.dma_start(out=out[:, 0:C], in_=xp[:]).then_inc(dma_sem, 16)
            sync.dma_start(out=out[:, C : 2 * C], in_=x[:]).then_inc(dma_sem, 16)
            sync.wait_ge(dma_sem, 32)

    return nc


def _get_nc():
    if "nc" not in _CACHE:
        _CACHE["nc"] = _build_nc()
    return _CACHE["nc"]


def _shard_inputs(x, buffer, size, head):
    x = np.ascontiguousarray(np.asarray(x), dtype=np.float32)
    assert x.shape == (B, C)
    d = np.asarray(buffer).shape[0]
    full = int(np.asarray(size)) == d
    if full:
        xp = np.ascontiguousarray(np.asarray(buffer[int(np.asarray(head))]),
                                  dtype=np.float32)
    else:
        xp = np.zeros((B, C), dtype=np.float32)
    return [
        {
            "xp": xp[i * ROWS : (i + 1) * ROWS],
            "x": x[i * ROWS : (i + 1) * ROWS],
        }
        for i in range(N_CORES)
    ]


def _run(in_maps, **kw):
    from concourse.bass_utils import run_bass_kernel_spmd

    return run_bass_kernel_spmd(_get_nc(), in_maps, list(range(N_CORES)), **kw)


def kernel(x, buffer, size, head):
    in_maps = _shard_inputs(x, buffer, size, head)
    res = _run(in_maps)
    return np.concatenate([res.results[i]["out"] for i in range(N_CORES)], axis=0)


def kernel_profiled(x, buffer, size, head, **kw):
    """Like kernel() but also returns BassKernelResults (exec_time_ns etc.)."""
    in_maps = _shard_inputs(x, buffer, size, head)
    res = _run(in_maps, trace=True, **kw)
    out = np.concatenate([res.results[i]["out"] for i in range(N_CORES)], axis=0)
    return out, res


# revision 2
# speedup vs baseline: 1.1713x; 1.1713x over previous
"""CausalQueue concat kernel for Trainium2 (8 NeuronCores, SPMD).

Semantics (from the reference):
    x_past = buffer[head] if size == D else zeros_like(x)
    out    = concat((x_past, x), axis=1)          # [B, 2*C]

Strategy: the op is embarrassingly data-parallel over the batch axis.
The host slices the single needed ring-buffer row `buffer[head]` (4 MB
out of the 512 MB buffer) and shards batch rows across the 8 cores
(256 rows each). Each core performs two DRAM->DRAM DMAs writing the
two column-halves of its output shard. Per-core HBM traffic: 1 MB read
+ 1 MB write.
"""

import numpy as np

B, C, D = 2048, 512, 128
N_CORES = 8
ROWS = B // N_CORES  # 256

_CACHE: dict = {}


import os

_VARIANT = os.environ.get("CQ_VARIANT", "dual")


def _build_nc():
    import concourse.bass as bass
    import concourse.mybir as mybir

    if _VARIANT == "base":
        nc = bass.Bass()
    else:
        nc = bass.Bass(enable_partition_id=False, monotonic_sem_count=0)
    xp = nc.declare_dram_parameter("xp", [ROWS, C], mybir.dt.float32, isOutput=False)
    x = nc.declare_dram_parameter("x", [ROWS, C], mybir.dt.float32, isOutput=False)
    out = nc.declare_dram_parameter(
        "out", [ROWS, 2 * C], mybir.dt.float32, isOutput=True
    )

    with (
        nc.Block() as block,
        nc.semaphore("dma_sem") as dma_sem,
    ):
        if _VARIANT in ("base", "sync2"):
            # both DMAs on the Sync HWDGE ring
            @block.sync
            def _(sync):
                sync.dma_start(out=out[:, 0:C], in_=xp[:]).then_inc(dma_sem, 16)
                sync.dma_start(out=out[:, C : 2 * C], in_=x[:]).then_inc(dma_sem, 16)
                sync.wait_ge(dma_sem, 32)

        else:  # "dual": one DMA per HWDGE ring (Sync + Act)
            @block.scalar
            def _(scalar):
                scalar.dma_start(out=out[:, C : 2 * C], in_=x[:]).then_inc(dma_sem, 16)

            @block.sync
            def _(sync):
                sync.dma_start(out=out[:, 0:C], in_=xp[:]).then_inc(dma_sem, 16)
                sync.wait_ge(dma_sem, 32)

    return nc


def _get_nc():
    if "nc" not in _CACHE:
        _CACHE["nc"] = _build_nc()
    return _CACHE["nc"]


def _shard_inputs(x, buffer, size, head):
    x = np.ascontiguousarray(np.asarray(x), dtype=np.float32)
    assert x.shape == (B, C)
    d = np.asarray(buffer).shape[0]
    full = int(np.asarray(size)) == d
    if full:
        xp = np.ascontiguousarray(np.asarray(buffer[int(np.asarray(head))]),
                                  dtype=np.float32)
    else:
        xp = np.zeros((B, C), dtype=np.float32)
    return [
        {
            "xp": xp[i * ROWS : (i + 1) * ROWS],
            "x": x[i * ROWS : (i + 1) * ROWS],
        }
        for i in range(N_CORES)
    ]


def _run(in_maps, **kw):
    from concourse.bass_utils import run_bass_kernel_spmd

    return run_bass_kernel_spmd(_get_nc(), in_maps, list(range(N_CORES)), **kw)


def kernel(x, buffer, size, head):
    in_maps = _shard_inputs(x, buffer, size, head)
    res = _run(in_maps)
    return np.concatenate([res.results[i]["out"] for i in range(N_CORES)], axis=0)


def kernel_profiled(x, buffer, size, head, **kw):
    """Like kernel() but also returns BassKernelResults (exec_time_ns etc.)."""
    in_maps = _shard_inputs(x, buffer, size, head)
    res = _run(in_maps, trace=True, **kw)
    out = np.concatenate([res.results[i]["out"] for i in range(N_CORES)], axis=0)
    return out, res
